# revision 50
# baseline (speedup 1.0000x reference)
"""Batched multi-head graph attention (GAT) kernel for 8 Trainium2 NeuronCores.

Math (per batch b, head h):
    hp      = h[b] @ w[h]                          # [N, F]
    t       = tanh(hp)
    s       = t @ a_src[h];  d = t @ a_dst[h]      # [N]
    score   = leaky_relu(s_i + d_j, 0.2)
    e       = where(adj>0, exp(score), 0)
    out     = (e / e.sum(-1, keepdim)) @ hp + bias

Key identities used on-device:
    exp(leaky(z)) = max(exp(z), exp(0.2 z))                      (slope < 1)
                  = e^{0.2 s_i} * max(e^{0.8 s_i} e^{d_j}, e^{0.2 d_j})
    The e^{0.2 s_i} factor is constant along j, so it cancels in the
    softmax ratio.  With q=e^{0.8s}, v=e^d, v2=e^{0.2d} the masked weight is
        D[j,i] = adj[i,j] * max(q_i v_j, v2_j)     (up to a row-constant)
    computed per 128x1024 tile as ONE dual-scalar op + ONE mask multiply
    (fp16).  A single PE matmul against lhsT=[hp | 1] accumulates numerator
    and denominator together into PSUM [65, 512].

adj mask trick: adj values are exactly 0.0/1.0 fp32 = 0x00000000/0x3F800000.
The low 16 bits are zero, and the high 16 bits (0x3F80) read as fp16 equal
1.875 -- a constant scale on every surviving softmax term, which cancels in
the normalization.  So the host passes the high uint16 halves (half the
bytes) and the device DMA-transposes them directly as the fp16 mask.

Sharding: 8 cores = 4 batches x 2 query-row halves; each core handles all 4
heads for its 1024 query rows against all 2048 keys.  Keys are rotated on
host so each core's queries are local rows [0, 1024).
"""

import os
from contextlib import ExitStack

import numpy as np

import concourse.bass as bass
import concourse.mybir as mybir
import concourse.tile as tile
from concourse import bacc
from concourse.bass_utils import run_bass_kernel_spmd
from concourse.masks import make_identity

F32 = mybir.dt.float32
F16 = mybir.dt.float16
U16 = mybir.dt.uint16
ALU = mybir.AluOpType
ACTF = mybir.ActivationFunctionType
AX = mybir.AxisListType

B, N, H, F = 4, 2048, 4, 64
NCORES = 8
ROWS = N // 2          # query rows per core
KEYS = N               # keys per core (full)
NEG_SLOPE = 0.2


def default_assign(jb, h):
    """E-tile source for head h.

    "dve": E = max(q*v, v2) via one DVE tensor_scalar (full weight).
    "act": E = relu(q*v - v2) via one ACT pass; the missing v2*adjT
           contribution is added by an extra matmul whose weights are
           the v2-scaled [hp | 1] (exact, since adjT >= 0).
    """
    return "act" if h % 2 == 1 else "dve"


def build_program(rows=ROWS, keys=KEYS, heads=H, f=F, assign=default_assign,
                  sd_engine="vector"):
    nc = bacc.Bacc("TRN2", target_bir_lowering=False, debug=False)

    kb = keys // 128          # key blocks
    qb = rows // 128          # query blocks
    nhalf = rows // 512       # output column halves (psum tiles per head)
    fe = f + 1                # hp with ones column appended

    hb_d = nc.dram_tensor("hb", [keys, f], F32, kind="ExternalInput")
    adjh_d = nc.dram_tensor("adjh", [rows, keys], U16, kind="ExternalInput")
    w_d = nc.dram_tensor("wmat", [heads, f, f], F32, kind="ExternalInput")
    ap_d = nc.dram_tensor("apairt", [heads, 2, f], F32, kind="ExternalInput")
    out_d = nc.dram_tensor("out", [heads, rows, f], F32,
                           kind="ExternalOutput")

    eng = {"dve": nc.vector, "gps": nc.gpsimd}
    sd_eng = nc.vector if sd_engine == "vector" else nc.gpsimd

    with tile.TileContext(nc) as tc:
        with (
            tc.tile_pool(name="const", bufs=1) as const,
            tc.tile_pool(name="persist", bufs=1) as persist,
            tc.tile_pool(name="stmp", bufs=4) as stmp,
        ):
            id16 = const.tile([128, 128], F16, tag="id16")
            make_identity(nc, id16)
            id32 = const.tile([128, 128], F32, tag="id32")
            make_identity(nc, id32)

            # ---- global loads -------------------------------------------
            h32 = persist.tile([128, kb, f], F32, tag="h32")
            nc.sync.dma_start(
                out=h32, in_=hb_d.ap().rearrange("(t p) f -> p t f", p=128))
            h16 = persist.tile([128, kb, f], F16, tag="h16")
            nc.vector.tensor_copy(h16, h32)

            w32 = persist.tile([f, heads, f], F32, tag="w32")
            nc.sync.dma_start(out=w32, in_=w_d.ap().rearrange("h f o -> f h o"))
            w16 = persist.tile([f, heads, f], F16, tag="w16")
            nc.vector.tensor_copy(w16, w32)

            apr32 = persist.tile([1, heads, 2, f], F32, tag="apr32")
            nc.sync.dma_start(out=apr32, in_=ap_d.ap().unsqueeze(0))
            abc32 = persist.tile([128, heads, 2, f], F32, tag="abc32")
            nc.gpsimd.partition_broadcast(abc32, apr32)
            a16 = persist.tile([128, heads, 2, f], F16, tag="a16")
            nc.vector.tensor_copy(a16, abc32)

            # ---- hT (transposed h, fp16) --------------------------------
            hT16 = persist.tile([64, keys], F16, tag="hT16")
            g_ht = min(4, kb)
            with tc.tile_pool(name="psum_ht", bufs=2, space="PSUM") as pht:
                for g in range(kb // g_ht):
                    pt = pht.tile([64, g_ht * 128], F16, tag="pht")
                    for t in range(g_ht):
                        blk = g * g_ht + t
                        nc.tensor.transpose(
                            pt[:, t * 128:(t + 1) * 128],
                            h16[:, blk, :], id16)
                    nc.vector.tensor_copy(
                        hT16[:, g * g_ht * 128:(g + 1) * g_ht * 128], pt)

            # ---- per-head setup -----------------------------------------
            hpt = []   # [128, kb, fe] fp16 -- [hp | 1] in key-block layout
            qbc = []   # [128, rows] fp16 -- exp(0.8 s_i) broadcast
            vv, vv2, nvv2 = [], [], []
            hpt2 = {}  # v2-scaled [hp | 1] for "act"-path heads
            with (
                tc.tile_pool(name="psum_hp", bufs=2, space="PSUM") as php,
                tc.tile_pool(name="psum_q", bufs=2, space="PSUM") as pq,
            ):
                for h in range(heads):
                    hpt_h = persist.tile([128, kb, fe], F16, tag=f"hpt{h}")
                    tanh_h = stmp.tile([128, kb, f], F16, tag="tanh")
                    g_hp = min(8, kb)
                    for k in range(kb // g_hp):
                        pp = php.tile([128, g_hp * f], F32, tag="php")
                        for t in range(g_hp):
                            blk = k * g_hp + t
                            nc.tensor.matmul(
                                pp[:, t * f:(t + 1) * f],
                                lhsT=hT16[:, blk * 128:(blk + 1) * 128],
                                rhs=w16[:, h, :], start=True, stop=True)
                        nc.scalar.activation(
                            hpt_h[:, k * g_hp:(k + 1) * g_hp, 0:f],
                            pp.rearrange("p (t o) -> p t o", o=f),
                            ACTF.Identity)
                        nc.scalar.activation(
                            tanh_h[:, k * g_hp:(k + 1) * g_hp, :],
                            pp.rearrange("p (t o) -> p t o", o=f),
                            ACTF.Tanh)
                    nc.vector.memset(hpt_h[:, :, f:fe], 1.0)
                    hpt.append(hpt_h)

                    # s, d via elementwise mul + per-block reduce
                    prod = stmp.tile([128, kb, 2, f], F16, tag="prod")
                    sd_eng.tensor_tensor(
                        out=prod,
                        in0=tanh_h.unsqueeze(2).broadcast_to([128, kb, 2, f]),
                        in1=a16[:, h].unsqueeze(1).broadcast_to(
                            [128, kb, 2, f]),
                        op=ALU.mult)
                    # split the reduction: the s-part (first qb blocks) is
                    # all the q-chain needs, so it unblocks q/broadcast
                    # before the full d reduction finishes
                    sums = stmp.tile([128, kb, 2], F32, tag="sums")
                    sd_eng.reduce_sum(sums[:, 0:qb, 0:1],
                                      prod[:, 0:qb, 0:1, :], axis=AX.X)
                    sd_eng.reduce_sum(sums[:, :, 1:2],
                                      prod[:, :, 1:2, :], axis=AX.X)

                    v_h = persist.tile([128, kb], F32, tag=f"v{h}")
                    v2_h = persist.tile([128, kb], F32, tag=f"v2{h}")
                    nv2_h = persist.tile([128, kb], F32, tag=f"nv2{h}")
                    nc.scalar.activation(v_h, sums[:, :, 1], ACTF.Exp)
                    nc.scalar.activation(v2_h, sums[:, :, 1], ACTF.Exp,
                                         scale=NEG_SLOPE)
                    nc.vector.tensor_scalar_mul(nv2_h, v2_h, -1.0)
                    vv.append(v_h)
                    vv2.append(v2_h)
                    nvv2.append(nv2_h)

                    if assign(0, h) == "act":
                        # v2-scaled [hp | 1]: weights for the matmul that
                        # restores the v2*adjT part of max(qv, v2)*adjT
                        hpt2_h = persist.tile([128, kb, fe], F16,
                                              tag=f"hpt2{h}")
                        nc.vector.tensor_tensor(
                            out=hpt2_h, in0=hpt_h,
                            in1=v2_h.unsqueeze(2).broadcast_to(
                                [128, kb, fe]),
                            op=ALU.mult)
                        hpt2[h] = hpt2_h

                    # q = exp(0.8 s) over this core's query rows, broadcast
                    pq_t = pq.tile([1, rows], F32, tag="pq")
                    for t in range(qb):
                        nc.tensor.transpose(
                            pq_t[:, t * 128:(t + 1) * 128],
                            sums[:, t:t + 1, 0:1], id32)
                    qrow = stmp.tile([1, rows], F16, tag="qrow")
                    nc.scalar.activation(qrow, pq_t, ACTF.Exp,
                                         scale=1.0 - NEG_SLOPE)
                    qb_h = persist.tile([128, rows], F16, tag=f"qb{h}")
                    nc.gpsimd.partition_broadcast(qb_h, qrow)
                    qbc.append(qb_h)

            # ---- main loop: masked weights + fused matmul ----------------
            # heads run in PAIRS so pair 2's setup overlaps pair 1's loop
            # and pair 1's normalize/store overlaps pair 2's loop.
            nacc = heads * nhalf
            acc_sb = persist.tile([fe, nacc, 512], F32, tag="acc_sb")
            pairs = [tuple(range(p, min(p + 2, heads)))
                     for p in range(0, heads, 2)]
            nq = 512 // 128  # transpose chunks per acc tile

            with (
                tc.tile_pool(name="adjp", bufs=kb) as adjp,
                tc.tile_pool(name="ep", bufs=4) as ep,
                tc.tile_pool(name="dp", bufs=4) as dp,
                tc.tile_pool(name="outp", bufs=4) as outp,
                ExitStack() as pools,
            ):
                # prefetch ALL transposed mask blocks up front (resident)
                adjts = []
                for jb in range(kb):
                    adjt = adjp.tile([128, rows], U16, tag="adjt",
                                     name=f"adjt{jb}")
                    nc.sync.dma_start_transpose(
                        adjt, adjh_d.ap()[:, jb * 128:(jb + 1) * 128])
                    adjts.append(adjt)

                # PSUM pools are stack-allocated: open acc pools in reverse
                # pair order so pair 0's closes first (LIFO), letting its
                # normalize PSUM reuse those banks while pair 1 still
                # accumulates in its own.
                acc_stacks = {}
                accps = {}
                for pi in reversed(range(len(pairs))):
                    st = ExitStack()
                    acc_stacks[pi] = st
                    accps[pi] = st.enter_context(
                        tc.tile_pool(name=f"accp{pi}", bufs=1, space="PSUM"))
                accs = {}
                for pi, pair in enumerate(pairs):
                    for h in pair:
                        for half in range(nhalf):
                            i = h * nhalf + half
                            accs[i] = accps[pi].tile(
                                [fe, 512], F32, tag=f"acc{i}",
                                name=f"acc{i}")

                for pi, pair in enumerate(pairs):
                    np_ = len(pair)
                    for jb in range(kb):
                        adj16 = adjts[jb].bitcast(F16)
                        ea = ep.tile([128, np_, rows], F16, tag="ea")
                        da = dp.tile([128, np_, rows], F16, tag="da")
                        for k, h in enumerate(pair):
                            v_s = vv[h][:, jb:jb + 1]
                            v2_s = vv2[h][:, jb:jb + 1]
                            if assign(jb, h) == "act":
                                nc.scalar.activation(
                                    ea[:, k, :], qbc[h], ACTF.Relu,
                                    bias=nvv2[h][:, jb:jb + 1], scale=v_s)
                            else:
                                nc.vector.tensor_scalar(
                                    out=ea[:, k, :], in0=qbc[h],
                                    scalar1=v_s, scalar2=v2_s,
                                    op0=ALU.mult, op1=ALU.max)
                        nc.vector.tensor_tensor(
                            out=da, in0=ea,
                            in1=adj16.unsqueeze(1).broadcast_to(
                                [128, np_, rows]),
                            op=ALU.mult)
                        dsrc = da
                        for k, h in enumerate(pair):
                            is_act = assign(jb, h) == "act"
                            for half in range(nhalf):
                                rhs_slice = slice(half * 512,
                                                  (half + 1) * 512)
                                nc.tensor.matmul(
                                    accs[h * nhalf + half],
                                    lhsT=hpt[h][:, jb, :],
                                    rhs=dsrc[:, k, rhs_slice],
                                    start=(jb == 0),
                                    stop=(jb == kb - 1 and not is_act))
                                if is_act:
                                    nc.tensor.matmul(
                                        accs[h * nhalf + half],
                                        lhsT=hpt2[h][:, jb, :],
                                        rhs=adj16[:, rhs_slice],
                                        start=False, stop=(jb == kb - 1))

                    # spill this pair's accumulators to SBUF (PE reads SBUF
                    # only); alternate engines so copies drain in parallel
                    for h in pair:
                        for half in range(nhalf):
                            i = h * nhalf + half
                            if i % 2 == 0:
                                nc.scalar.activation(
                                    acc_sb[:, i, :], accs[i], ACTF.Identity)
                            else:
                                nc.vector.tensor_copy(
                                    acc_sb[:, i, :], accs[i])
                    # free this pair's PSUM banks, then normalize this pair
                    # in transposed [i, o] form (overlaps next pair's loop)
                    acc_stacks[pi].close()
                    ptf_st = ExitStack()
                    ptf = ptf_st.enter_context(
                        tc.tile_pool(name=f"ptf{pi}", bufs=2, space="PSUM"))
                    for h in pair:
                        for half in range(nhalf):
                            i = h * nhalf + half
                            pt = ptf.tile([128, nq, fe], F32, tag=f"pt{pi}")
                            for q in range(nq):
                                nc.tensor.transpose(
                                    pt[:, q, :],
                                    acc_sb[:, i, q * 128:(q + 1) * 128],
                                    id32[0:fe, 0:fe])
                            rcol = outp.tile([128, nq], F32, tag="rcol")
                            nc.vector.reciprocal(rcol, pt[:, :, f])
                            osb = outp.tile([128, nq, f], F32, tag="osb")
                            nc.vector.tensor_tensor(
                                out=osb, in0=pt[:, :, 0:f],
                                in1=rcol.unsqueeze(2).broadcast_to(
                                    [128, nq, f]),
                                op=ALU.mult)
                            nc.sync.dma_start(
                                out=out_d.ap()[
                                    h, half * 512:(half + 1) * 512, :]
                                .rearrange("(q p) f -> p q f", p=128),
                                in_=osb)
                    ptf_st.close()
    nc.compile()
    return nc


_PROGRAM_CACHE = {}


def _get_program():
    key = "full"
    if key not in _PROGRAM_CACHE:
        _PROGRAM_CACHE[key] = build_program()
    return _PROGRAM_CACHE[key]


def make_in_maps(h, adj, w, a_src, a_dst):
    """Shard + marshal the full inputs into 8 per-core input maps."""
    h = np.ascontiguousarray(np.asarray(h, dtype=np.float32))
    adj = np.ascontiguousarray(np.asarray(adj, dtype=np.float32))
    w = np.ascontiguousarray(np.asarray(w, dtype=np.float32))
    apairt = np.ascontiguousarray(
        np.concatenate([np.asarray(a_src)[:, None, :, 0],
                        np.asarray(a_dst)[:, None, :, 0]],
                       axis=1).astype(np.float32))  # [H, 2, F]
    in_maps = []
    for c in range(NCORES):
        b, r0 = c // 2, (c % 2) * ROWS
        hb = np.concatenate([h[b, r0:], h[b, :r0]], axis=0)  # rotate keys
        adj_rows = adj[b, r0:r0 + ROWS]
        adj_rot = np.concatenate([adj_rows[:, r0:], adj_rows[:, :r0]], axis=1)
        adjh = np.ascontiguousarray(
            adj_rot.view(np.uint16).reshape(ROWS, KEYS, 2)[:, :, 1])
        in_maps.append({
            "hb": np.ascontiguousarray(hb),
            "adjh": adjh,
            "wmat": w,
            "apairt": apairt,
        })
    return in_maps


def assemble_output(results, bias):
    """Gather per-core [H, ROWS, F] results into [B, H, N, F]."""
    out = np.empty((B, H, N, F), dtype=np.float32)
    for c in range(NCORES):
        b, r0 = c // 2, (c % 2) * ROWS
        out[b, :, r0:r0 + ROWS, :] = results[c]["out"]
    if bias is not None:
        out = out + np.asarray(bias, dtype=np.float32)[None, None, None, :]
    return out


def run(h, adj, w, a_src, a_dst, bias, trace=False, trace_kwargs=None):
    nc = _get_program()
    in_maps = make_in_maps(h, adj, w, a_src, a_dst)
    res = run_bass_kernel_spmd(nc, in_maps, core_ids=list(range(NCORES)),
                               trace=trace, **(trace_kwargs or {}))
    return assemble_output(res.results, bias), res


def kernel(h, adj, w, a_src, a_dst, bias):
    out, _ = run(h, adj, w, a_src, a_dst, bias,
                 trace=bool(int(os.environ.get("GAT_TRACE", "0"))))
    return out
